# revision 7
# baseline (speedup 1.0000x reference)
"""Trainium2 Bass kernel: 3x3 same-padding conv2d (NCHW), 32x64x112x112 input,
128 output channels, f32.

Strategy:
- Data-parallel over batch: 32 images -> 8 NeuronCores x 4 images.
- Per core, implicit-GEMM conv on TensorE: for each 3x3 tap, a matmul with
  contraction over the 64 input channels accumulates into PSUM. Two images are
  packed side by side in the 128-partition contraction dim (partitions 0-63 =
  image A channels, 64-127 = image B channels) and computed concurrently via
  PE row-group tiling (tile_position (0,0) vs (64,0)).
- The input is zero-padded to 114x114 on host, so every tap is a plain
  shifted access pattern in SBUF and the input DMA is fully contiguous.
- float32r matmuls (fp32 data, reduced-precision PE multiply) run at 1
  cycle/row vs fp32's 4 — set USE_F32R=False for exact-fp32 fallback.
- PSUM is evacuated via VectorE (image A) and ScalarE (image B) into SBUF
  staging tiles of 16 output rows, then DMA'd to HBM in ~0.9MB transfers.

Weights are pre-transposed on host to [c_dup(128), tap(9), f(128)] so the
matmul lhsT (stationary [K=64, M=128]) is a direct slice.
"""

import sys

if "/opt/trn_rl_repo" not in sys.path:
    sys.path.insert(0, "/opt/trn_rl_repo")

import json

import numpy as np

B, C, H, W = 32, 64, 112, 112
F = 128
KH = KW = 3
NCORES = 8
BPC = B // NCORES  # images per core
HP, WP = H + 2, W + 2  # zero-padded plane
RT = 4  # output rows per PSUM tile (448 = 4*112 <= 512 fp32 bank limit)
RG = 16  # output rows per SBUF staging tile / output DMA
USE_F32R = True

# ---------------------------------------------------------------------------
# Sync-split post-pass: this container's walrus build rejects instructions
# with >1 (DMA) / >2 (other) sem waits or >1 sem update (Tile attaches up to
# 27 waits to its tail drain). Move excess waits onto standalone
# EventSemaphore instructions before the instruction, and excess updates onto
# ones after it. DMA completion updates cannot be moved (they fire from the
# SDMA engines), so assert there.
# ---------------------------------------------------------------------------
_MAXU = 1
_split_ctr = [0]


def _max_waits(opcode: str) -> int:
    # Observed walrus limits: EventSemaphore takes 2 waits; DMACopy and
    # Matmult (whose waits land on the lowered LDWEIGHTS) take only 1.
    # Default to 1 everywhere and pack the split-off waits 2-per-EventSem.
    return 2 if opcode == "EventSemaphore" else 1


def _evsem(engine, debug, waits, updates):
    _split_ctr[0] += 1
    return {
        "debug": debug,
        "engine": engine,
        "ins": [],
        "name": f"syncsplit_{_split_ctr[0]}",
        "opcode": "EventSemaphore",
        "outs": [],
        "sync_info": {"on_update": updates, "on_wait": waits},
    }


def _split_sync_json(bir_bytes: bytes) -> bytes:
    js = json.loads(bir_bytes)
    for f in js["functions"]:
        for b in f["blocks"]:
            out = []
            for inst in b["instructions"]:
                si = inst.get("sync_info")
                waits = (si.get("on_wait") or []) if si else []
                updates = (si.get("on_update") or []) if si else []
                eng = inst.get("engine")
                dbg = inst.get("debug", 0)
                if len(waits) > _max_waits(inst.get("opcode")):
                    for i in range(0, len(waits), 2):
                        out.append(_evsem(eng, dbg, waits[i : i + 2], []))
                    si["on_wait"] = []
                extra_updates = []
                if len(updates) > _MAXU:
                    assert inst.get("opcode") != "DMACopy", (
                        f"DMA {inst.get('name')} has {len(updates)} updates; "
                        "cannot split without changing completion semantics"
                    )
                    extra_updates = updates[_MAXU:]
                    si["on_update"] = updates[:_MAXU]
                out.append(inst)
                for i in range(0, len(extra_updates), _MAXU):
                    out.append(_evsem(eng, dbg, [], extra_updates[i : i + _MAXU]))
            b["instructions"] = out
    return json.dumps(js).encode()


def _patch_nc(nc):
    orig = nc.to_json_bytes
    nc.to_json_bytes = lambda: _split_sync_json(orig())
    return nc


# ---------------------------------------------------------------------------
# Kernel build
# ---------------------------------------------------------------------------
_NC_CACHE = {}


def _build_nc():
    import concourse.bass as bass
    import concourse.mybir as mb
    import concourse.tile as tile

    f32 = mb.dt.float32
    dt_in = mb.dt.float32r if USE_F32R else f32

    nc = bass.Bass("TRN2", target_bir_lowering=False)
    x = nc.dram_tensor("x", [BPC, C, HP, WP], dt_in, kind="ExternalInput")
    w = nc.dram_tensor("w", [2 * C, KH * KW, F], dt_in, kind="ExternalInput")
    y = nc.dram_tensor("y", [BPC, F, H, W], f32, kind="ExternalOutput")

    n_pairs = BPC // 2
    n_groups = H // RG
    rt_per_g = RG // RT

    with tile.TileContext(nc) as tc:
        with (
            tc.tile_pool(name="wpool", bufs=1) as wpool,
            tc.tile_pool(name="xpool", bufs=2) as xpool,
            tc.tile_pool(name="spool", bufs=2) as spool,
            tc.tile_pool(name="ppool", bufs=3, space="PSUM") as ppool,
        ):
            w_sb = wpool.tile([2 * C, KH * KW, F], dt_in)
            nc.sync.dma_start(out=w_sb[:], in_=w[:])

            for p in range(n_pairs):
                xbuf = xpool.tile([2 * C, HP, WP], dt_in)
                nc.sync.dma_start(
                    out=xbuf[:],
                    in_=x[2 * p : 2 * p + 2].rearrange("b c h w -> (b c) h w"),
                )
                for g in range(n_groups):
                    stage0 = spool.tile([F, RG, W], f32)
                    stage1 = spool.tile([F, RG, W], f32)
                    for rt in range(rt_per_g):
                        y0 = g * RG + rt * RT
                        ps0 = ppool.tile([F, RT, W], f32)
                        ps1 = ppool.tile([F, RT, W], f32)
                        for tap in range(KH * KW):
                            dy, dx = divmod(tap, KW)
                            first, last = tap == 0, tap == KH * KW - 1
                            nc.tensor.matmul(
                                ps0[:],
                                lhsT=w_sb[0:C, tap, :],
                                rhs=xbuf[0:C, y0 + dy : y0 + dy + RT, dx : dx + W],
                                start=first,
                                stop=last,
                            )
                            nc.tensor.matmul(
                                ps1[:],
                                lhsT=w_sb[C : 2 * C, tap, :],
                                rhs=xbuf[
                                    C : 2 * C,
                                    y0 + dy : y0 + dy + RT,
                                    dx : dx + W,
                                ],
                                start=first,
                                stop=last,
                            )
                        nc.vector.tensor_copy(
                            out=stage0[:, rt * RT : (rt + 1) * RT, :], in_=ps0[:]
                        )
                        nc.scalar.copy(
                            out=stage1[:, rt * RT : (rt + 1) * RT, :], in_=ps1[:]
                        )
                    nc.sync.dma_start(
                        out=y[2 * p, :, g * RG : (g + 1) * RG, :], in_=stage0[:]
                    )
                    nc.sync.dma_start(
                        out=y[2 * p + 1, :, g * RG : (g + 1) * RG, :], in_=stage1[:]
                    )
    _patch_nc(nc)
    return nc


def _get_nc():
    if "nc" not in _NC_CACHE:
        _NC_CACHE["nc"] = _build_nc()
    return _NC_CACHE["nc"]


def _prep_weights(filters: np.ndarray) -> np.ndarray:
    # [F, C, KH, KW] -> [C, KH*KW, F], duplicated on the partition dim so the
    # second image's matmuls (partitions 64-127) have their own copy.
    wts = np.ascontiguousarray(
        np.asarray(filters, dtype=np.float32).transpose(1, 2, 3, 0)
    ).reshape(C, KH * KW, F)
    return np.ascontiguousarray(np.concatenate([wts, wts], axis=0))


def _prep_input(dataset: np.ndarray) -> np.ndarray:
    dataset = np.asarray(dataset, dtype=np.float32)
    xp = np.zeros((B, C, HP, WP), dtype=np.float32)
    xp[:, :, 1 : HP - 1, 1 : WP - 1] = dataset
    return xp


def run_sharded(dataset, filters, **kwargs):
    from concourse import bass_utils

    nc = _get_nc()
    xp = _prep_input(dataset)
    wts = _prep_weights(filters)
    in_maps = [
        {"x": xp[i * BPC : (i + 1) * BPC], "w": wts} for i in range(NCORES)
    ]
    res = bass_utils.run_bass_kernel_spmd(
        nc, in_maps, core_ids=list(range(NCORES)), **kwargs
    )
    out = np.concatenate([res.results[i]["y"] for i in range(NCORES)], axis=0)
    return out, res


def kernel(dataset, filters):
    out, _ = run_sharded(dataset, filters)
    return out


# revision 10
# speedup vs baseline: 1.2153x; 1.2153x over previous
"""Trainium2 Bass kernel: 3x3 same-padding conv2d (NCHW), 32x64x112x112 input,
128 output channels, f32.

Strategy:
- Data-parallel over batch: 32 images -> 8 NeuronCores x 4 images.
- Per core, implicit-GEMM conv on TensorE: for each 3x3 tap, a matmul with
  contraction over the 64 input channels accumulates into PSUM. Two images are
  packed side by side in the 128-partition contraction dim (partitions 0-63 =
  image A channels, 64-127 = image B channels) and computed concurrently via
  PE row-group tiling (tile_position (0,0) vs (64,0)).
- The input is zero-padded to 114x114 on host, so every tap is a plain
  shifted access pattern in SBUF and the input DMA is fully contiguous.
- float32r matmuls (fp32 data, reduced-precision PE multiply) run at 1
  cycle/row vs fp32's 4 — set USE_F32R=False for exact-fp32 fallback.
- PSUM is evacuated via VectorE (image A) and ScalarE (image B) into SBUF
  staging tiles of 16 output rows, then DMA'd to HBM in ~0.9MB transfers.

Weights are pre-transposed on host to [c_dup(128), tap(9), f(128)] so the
matmul lhsT (stationary [K=64, M=128]) is a direct slice.
"""

import sys

if "/opt/trn_rl_repo" not in sys.path:
    sys.path.insert(0, "/opt/trn_rl_repo")

import json

import numpy as np

B, C, H, W = 32, 64, 112, 112
F = 128
KH = KW = 3
NCORES = 8
BPC = B // NCORES  # images per core
HP, WP = H + 2, W + 2  # zero-padded plane
RT = 4  # output rows per PSUM tile (448 = 4*112 <= 512 fp32 bank limit)
RG = 16  # output rows per SBUF staging tile / output DMA
USE_F32R = True

# ---------------------------------------------------------------------------
# Sync-split post-pass: this container's walrus build rejects instructions
# with >1 (DMA) / >2 (other) sem waits or >1 sem update (Tile attaches up to
# 27 waits to its tail drain). Move excess waits onto standalone
# EventSemaphore instructions before the instruction, and excess updates onto
# ones after it. DMA completion updates cannot be moved (they fire from the
# SDMA engines), so assert there.
# ---------------------------------------------------------------------------
_MAXU = 1
_split_ctr = [0]


def _max_waits(opcode: str) -> int:
    # Observed walrus limits: EventSemaphore takes 2 waits; DMACopy and
    # Matmult (whose waits land on the lowered LDWEIGHTS) take only 1.
    # Default to 1 everywhere and pack the split-off waits 2-per-EventSem.
    return 2 if opcode == "EventSemaphore" else 1


def _evsem(engine, debug, waits, updates):
    _split_ctr[0] += 1
    return {
        "debug": debug,
        "engine": engine,
        "ins": [],
        "name": f"syncsplit_{_split_ctr[0]}",
        "opcode": "EventSemaphore",
        "outs": [],
        "sync_info": {"on_update": updates, "on_wait": waits},
    }


def _split_sync_json(bir_bytes: bytes) -> bytes:
    js = json.loads(bir_bytes)
    for f in js["functions"]:
        for b in f["blocks"]:
            out = []
            for inst in b["instructions"]:
                si = inst.get("sync_info")
                waits = (si.get("on_wait") or []) if si else []
                updates = (si.get("on_update") or []) if si else []
                eng = inst.get("engine")
                dbg = inst.get("debug", 0)
                if len(waits) > _max_waits(inst.get("opcode")):
                    for i in range(0, len(waits), 2):
                        out.append(_evsem(eng, dbg, waits[i : i + 2], []))
                    si["on_wait"] = []
                extra_updates = []
                if len(updates) > _MAXU:
                    assert inst.get("opcode") != "DMACopy", (
                        f"DMA {inst.get('name')} has {len(updates)} updates; "
                        "cannot split without changing completion semantics"
                    )
                    extra_updates = updates[_MAXU:]
                    si["on_update"] = updates[:_MAXU]
                out.append(inst)
                for i in range(0, len(extra_updates), _MAXU):
                    out.append(_evsem(eng, dbg, [], extra_updates[i : i + _MAXU]))
            b["instructions"] = out
    return json.dumps(js).encode()


def _patch_nc(nc):
    orig = nc.to_json_bytes
    nc.to_json_bytes = lambda: _split_sync_json(orig())
    return nc


# ---------------------------------------------------------------------------
# Kernel build
# ---------------------------------------------------------------------------
_NC_CACHE = {}


def _build_nc(repeat: int = 1):
    import contextlib

    import concourse.bass as bass
    import concourse.mybir as mb
    import concourse.tile as tile

    f32 = mb.dt.float32
    dt_in = mb.dt.float32r if USE_F32R else f32

    nc = bass.Bass("TRN2", target_bir_lowering=False)
    x = nc.dram_tensor("x", [BPC, C, HP, WP], dt_in, kind="ExternalInput")
    w = nc.dram_tensor("w", [2 * C, KH * KW, F], dt_in, kind="ExternalInput")
    y = nc.dram_tensor("y", [BPC, F, H, W], f32, kind="ExternalOutput")

    n_pairs = BPC // 2
    n_groups = H // RG
    rt_per_g = RG // RT

    with tile.TileContext(nc) as tc:
        with (
            tc.tile_pool(name="wpool", bufs=1) as wpool,
            tc.tile_pool(name="xpool", bufs=2) as xpool,
            tc.tile_pool(name="spool", bufs=2) as spool,
            tc.tile_pool(name="ppool", bufs=3, space="PSUM") as ppool,
        ):
            w_sb = wpool.tile([2 * C, KH * KW, F], dt_in)
            nc.sync.dma_start(out=w_sb[:], in_=w[:])

            # repeat>1 is a benchmarking mode: loop the whole computation on
            # device so NEFF exec time dominates per-call dispatch overhead.
            loop_ctx = (
                tc.For_i(0, repeat, 1) if repeat > 1 else contextlib.nullcontext()
            )
            with loop_ctx:
                _emit_body(
                    nc, tc, x, w_sb, y, xpool, spool, ppool, f32, dt_in,
                    n_pairs, n_groups, rt_per_g,
                )
    _patch_nc(nc)
    return nc


def _emit_body(
    nc, tc, x, w_sb, y, xpool, spool, ppool, f32, dt_in, n_pairs, n_groups, rt_per_g
):
    if True:
        if True:
            for p in range(n_pairs):
                xbuf = xpool.tile([2 * C, HP, WP], dt_in)
                nc.sync.dma_start(
                    out=xbuf[:],
                    in_=x[2 * p : 2 * p + 2].rearrange("b c h w -> (b c) h w"),
                )
                for g in range(n_groups):
                    stage0 = spool.tile([F, RG, W], f32)
                    stage1 = spool.tile([F, RG, W], f32)
                    for rt in range(rt_per_g):
                        y0 = g * RG + rt * RT
                        ps0 = ppool.tile([F, RT, W], f32)
                        ps1 = ppool.tile([F, RT, W], f32)
                        for tap in range(KH * KW):
                            dy, dx = divmod(tap, KW)
                            first, last = tap == 0, tap == KH * KW - 1
                            nc.tensor.matmul(
                                ps0[:],
                                lhsT=w_sb[0:C, tap, :],
                                rhs=xbuf[0:C, y0 + dy : y0 + dy + RT, dx : dx + W],
                                start=first,
                                stop=last,
                            )
                            nc.tensor.matmul(
                                ps1[:],
                                lhsT=w_sb[C : 2 * C, tap, :],
                                rhs=xbuf[
                                    C : 2 * C,
                                    y0 + dy : y0 + dy + RT,
                                    dx : dx + W,
                                ],
                                start=first,
                                stop=last,
                            )
                        nc.vector.tensor_copy(
                            out=stage0[:, rt * RT : (rt + 1) * RT, :], in_=ps0[:]
                        )
                        nc.scalar.copy(
                            out=stage1[:, rt * RT : (rt + 1) * RT, :], in_=ps1[:]
                        )
                    nc.sync.dma_start(
                        out=y[2 * p, :, g * RG : (g + 1) * RG, :], in_=stage0[:]
                    )
                    nc.sync.dma_start(
                        out=y[2 * p + 1, :, g * RG : (g + 1) * RG, :], in_=stage1[:]
                    )


def _get_nc(repeat: int = 1):
    if repeat not in _NC_CACHE:
        _NC_CACHE[repeat] = _build_nc(repeat)
    return _NC_CACHE[repeat]


def _prep_weights(filters: np.ndarray) -> np.ndarray:
    # [F, C, KH, KW] -> [C, KH*KW, F], duplicated on the partition dim so the
    # second image's matmuls (partitions 64-127) have their own copy.
    wts = np.ascontiguousarray(
        np.asarray(filters, dtype=np.float32).transpose(1, 2, 3, 0)
    ).reshape(C, KH * KW, F)
    return np.ascontiguousarray(np.concatenate([wts, wts], axis=0))


def _prep_input(dataset: np.ndarray) -> np.ndarray:
    dataset = np.asarray(dataset, dtype=np.float32)
    xp = np.zeros((B, C, HP, WP), dtype=np.float32)
    xp[:, :, 1 : HP - 1, 1 : WP - 1] = dataset
    return xp


def run_sharded(dataset, filters, repeat=1, **kwargs):
    from concourse import bass_utils

    nc = _get_nc(repeat)
    xp = _prep_input(dataset)
    wts = _prep_weights(filters)
    in_maps = [
        {"x": xp[i * BPC : (i + 1) * BPC], "w": wts} for i in range(NCORES)
    ]
    res = bass_utils.run_bass_kernel_spmd(
        nc, in_maps, core_ids=list(range(NCORES)), **kwargs
    )
    out = np.concatenate([res.results[i]["y"] for i in range(NCORES)], axis=0)
    return out, res


def kernel(dataset, filters):
    out, _ = run_sharded(dataset, filters)
    return out
